# revision 1
# baseline (speedup 1.0000x reference)
"""GraphSAGE (2-layer, mean aggregation) on 8 Trainium2 NeuronCores.

Strategy (per spec sharding_hint): destination nodes are sharded across the
8 cores (49 tiles of 128 nodes per core, LPT-balanced by degree so every
tile has nearly equal incoming-edge count). Edge lists are partitioned by
destination tile and padded to a uniform chunk count T per tile so one SPMD
program serves all cores. x and (between layers) h are replicated to every
core's HBM; per-edge source rows are fetched with indirect DMA gathers of
128 rows per instruction. The segment sum for a destination tile is built
on the PE: for each 128-edge chunk a 0/1 selection matrix S[e, n] =
(dst_slot[e] == n) is formed on the vector engine (iota + is_equal) and
S^T @ messages accumulates into PSUM over the tile's chunks. The mean
division, dense lin_l/lin_r matmuls, bias and ReLU all happen on-device;
layer-1 output h round-trips through the host (re-replication only, no
host float math on the compute path) and feeds the identical layer-2
program. All float tensor computation runs on the NeuronCores; the host
only does integer index preprocessing, sharding/layout, and un-sharding.
"""
import heapq
import sys
from contextlib import ExitStack

import numpy as np

for _p in ("/opt/trn_rl_repo",):
    if _p not in sys.path:
        sys.path.insert(0, _p)

import concourse.bass as bass
import concourse.tile as tile
from concourse import bacc, mybir
from concourse.bass_utils import run_bass_kernel_spmd
from concourse.masks import make_identity


def _ensure_axon_hooks():
    """run_bass_kernel_spmd(trace=True) imports antenv.axon_hooks, which this
    image lacks; install a ctypes-backed hook so tracing works (or degrades
    to a no-op instead of an ImportError)."""
    try:
        import antenv.axon_hooks  # noqa: F401
        return
    except ImportError:
        pass
    import contextlib
    import ctypes
    import types

    def _make_hook():
        try:
            lib = ctypes.CDLL("/opt/axon/libaxon_pjrt.so")
        except OSError:
            return None
        if not hasattr(lib, "axon_start_nrt_profile"):
            return None
        lib.axon_start_nrt_profile.argtypes = [ctypes.POINTER(ctypes.c_int64), ctypes.c_size_t]
        lib.axon_start_nrt_profile.restype = ctypes.c_int64
        lib.axon_stop_nrt_profile.argtypes = [ctypes.c_char_p]
        lib.axon_stop_nrt_profile.restype = ctypes.c_int64

        @contextlib.contextmanager
        def _hook(output_dir, device_ids):
            import jax
            jax.devices()
            if device_ids:
                ids = (ctypes.c_int64 * len(device_ids))(*device_ids)
                rc = lib.axon_start_nrt_profile(ids, len(device_ids))
            else:
                rc = lib.axon_start_nrt_profile(None, 0)
            if rc != 0:
                raise RuntimeError(f"axon_start_nrt_profile rc={rc}")
            try:
                yield
            finally:
                lib.axon_stop_nrt_profile(str(output_dir).encode())

        return _hook

    hook = _make_hook()
    mod = types.ModuleType("antenv.axon_hooks")
    mod.get_axon_ntff_profile_hook = lambda: hook
    mod.set_axon_ntff_profile_hook = lambda h: None
    import antenv
    antenv.axon_hooks = mod
    sys.modules["antenv.axon_hooks"] = mod


_ensure_axon_hooks()


def _run_spmd_retry(nc, in_maps, **kw):
    """One retry for transient NRT device errors (axon cores occasionally
    report EXEC_UNIT_UNRECOVERABLE right after a prior faulted run)."""
    import time
    try:
        return run_bass_kernel_spmd(nc, in_maps, core_ids=list(range(N_CORES)), **kw)
    except Exception:
        time.sleep(15)
        return run_bass_kernel_spmd(nc, in_maps, core_ids=list(range(N_CORES)), **kw)

N_NODES = 50000
N_EDGES = 800000
DIM_IN, DIM_H, DIM_OUT = 128, 256, 64
N_CORES = 8
P = 128
TILES_PER_CORE = 49                      # ceil(50000 / 8 / 128)
N_TILES = N_CORES * TILES_PER_CORE       # 392
NPAD_CORE = TILES_PER_CORE * P           # 6272
PAD_SLOT = 200.0                         # dst_rel sentinel: matches no iota lane

LAST_RESULTS = []   # test harness reads profiling results from here


def _partition_nodes(deg):
    """LPT-pack nodes into N_TILES bins of <=128 nodes, minimizing max bin
    degree-sum. Returns (tile_of, slot_of, T) with T = uniform chunks/tile."""
    order = np.argsort(-deg, kind="stable")
    heap = [(0, t) for t in range(N_TILES)]
    heapq.heapify(heap)
    counts = np.zeros(N_TILES, np.int64)
    sums = np.zeros(N_TILES, np.int64)
    tile_of = np.empty(N_NODES, np.int64)
    slot_of = np.empty(N_NODES, np.int64)
    for node in order:
        while True:
            s, t = heapq.heappop(heap)
            if counts[t] < P:
                break
        tile_of[node] = t
        slot_of[node] = counts[t]
        counts[t] += 1
        sums[t] += deg[node]
        if counts[t] < P:
            heapq.heappush(heap, (sums[t], t))
    T = int(np.ceil(sums.max() / P))
    return tile_of, slot_of, T


def _build_edge_layout(src, dst, tile_of, slot_of, T):
    """Per-core chunk-major index arrays.

    Returns src_cols, dst_cols: lists (per core) of [P, 49*T] arrays where
    column t*T + j holds chunk j of tile t: lane p is edge j*128+p of that
    tile's padded edge list (src node id / dst slot, PAD entries src=0,
    dst_rel=PAD_SLOT).
    """
    etile = tile_of[dst]
    order = np.argsort(etile, kind="stable")
    counts = np.bincount(etile, minlength=N_TILES)
    src_pad = np.zeros((N_TILES, T * P), np.int64)
    dst_pad = np.full((N_TILES, T * P), PAD_SLOT, np.float32)
    rank = np.arange(N_EDGES) - np.repeat(np.concatenate([[0], np.cumsum(counts)[:-1]]), counts)
    es, ed = src[order], dst[order]
    src_pad[etile[order], rank] = es
    dst_pad[etile[order], rank] = slot_of[ed]
    src_cols, dst_cols = [], []
    for c in range(N_CORES):
        sl = slice(c * TILES_PER_CORE, (c + 1) * TILES_PER_CORE)
        s = src_pad[sl].reshape(TILES_PER_CORE, T, P).transpose(2, 0, 1).reshape(P, TILES_PER_CORE * T)
        d = dst_pad[sl].reshape(TILES_PER_CORE, T, P).transpose(2, 0, 1).reshape(P, TILES_PER_CORE * T)
        src_cols.append(np.ascontiguousarray(s))
        dst_cols.append(np.ascontiguousarray(d))
    return src_cols, dst_cols


def _build_layer_program(T, n_table, f_in, f_out, relu):
    """One SAGE layer as an SPMD bass program.

    Inputs (per core): table [n_table, f_in] (gather source, replicated),
    selfT [f_in, NPAD_CORE] (own nodes' features, transposed),
    wlT/wrT packed [128, (f_in/128)*f_out], b_col [128, ceil(f_out/128)],
    src_idx int32 [P, 49*T], dst_rel f32 [P, 49*T], deg_col [P, 49].
    Output: outT [f_out, NPAD_CORE].
    """
    SI = f_in // P                       # contraction splits (1 or 2)
    SO = (f_out + P - 1) // P            # output-partition splits
    fo_sz = min(f_out, P)
    NCH = TILES_PER_CORE * T

    nc = bacc.Bacc("TRN2", target_bir_lowering=False, debug=False,
                   enable_asserts=False, num_devices=N_CORES)
    dt = mybir.dt
    table = nc.dram_tensor("table", [n_table, f_in], dt.float32, kind="ExternalInput").ap()
    selfTs = [nc.dram_tensor(f"selfT{si}", [P, NPAD_CORE], dt.float32, kind="ExternalInput").ap()
              for si in range(SI)]
    wlT = nc.dram_tensor("wlT", [P, SI * f_out], dt.float32, kind="ExternalInput").ap()
    wrT = nc.dram_tensor("wrT", [P, SI * f_out], dt.float32, kind="ExternalInput").ap()
    b_col = nc.dram_tensor("b_col", [P, SO], dt.float32, kind="ExternalInput").ap()
    src_idx = nc.dram_tensor("src_idx", [P, NCH], dt.int32, kind="ExternalInput").ap()
    dst_rel = nc.dram_tensor("dst_rel", [P, NCH], dt.float32, kind="ExternalInput").ap()
    deg_col = nc.dram_tensor("deg_col", [P, TILES_PER_CORE], dt.float32, kind="ExternalInput").ap()
    outT = nc.dram_tensor("outT", [f_out, NPAD_CORE], dt.float32, kind="ExternalOutput").ap()

    with tile.TileContext(nc) as tc:
        with ExitStack() as ctx:
            const = ctx.enter_context(tc.tile_pool(name="const", bufs=1))
            msgp = ctx.enter_context(tc.tile_pool(name="msgp", bufs=2))
            sp = ctx.enter_context(tc.tile_pool(name="sp", bufs=2))
            work = ctx.enter_context(tc.tile_pool(name="work", bufs=2))
            outp = ctx.enter_context(tc.tile_pool(name="outp", bufs=3))
            psA = ctx.enter_context(tc.tile_pool(name="psA", bufs=2, space="PSUM"))
            psB = ctx.enter_context(tc.tile_pool(name="psB", bufs=2, space="PSUM"))
            psC = ctx.enter_context(tc.tile_pool(name="psC", bufs=2, space="PSUM"))

            idx_sb = const.tile([P, NCH], dt.int32)
            nc.sync.dma_start(idx_sb[:], src_idx[:, :])
            dr_sb = const.tile([P, NCH], dt.float32)
            nc.sync.dma_start(dr_sb[:], dst_rel[:, :])
            deg_sb = const.tile([P, TILES_PER_CORE], dt.float32)
            nc.sync.dma_start(deg_sb[:], deg_col[:, :])
            wl_sb = const.tile([P, SI * f_out], dt.float32)
            nc.sync.dma_start(wl_sb[:], wlT[:, :])
            wr_sb = const.tile([P, SI * f_out], dt.float32)
            nc.sync.dma_start(wr_sb[:], wrT[:, :])
            b_sb = const.tile([P, SO], dt.float32)
            nc.sync.dma_start(b_sb[:], b_col[:, :])
            self_sb = []
            for si in range(SI):
                t_ = const.tile([P, NPAD_CORE], dt.float32, name=f"self_sb{si}")
                nc.sync.dma_start(t_[:], selfTs[si][:, :])
                self_sb.append(t_)

            ident = const.tile([P, P], dt.float32)
            make_identity(nc, ident[:])
            iota_sm = const.tile([P, P], dt.float32)
            nc.gpsimd.iota(iota_sm[:], pattern=[[1, P]], base=0, channel_multiplier=0,
                           allow_small_or_imprecise_dtypes=True)
            iota_big = const.tile([P, T * P], dt.float32)
            for _j in range(T):
                nc.vector.tensor_copy(iota_big[:, _j * P:(_j + 1) * P], iota_sm[:])

            recip = const.tile([P, TILES_PER_CORE], dt.float32)
            nc.vector.tensor_scalar_max(recip[:], deg_sb[:], 1.0)
            nc.vector.reciprocal(recip[:], recip[:])

            for t in range(TILES_PER_CORE):
                c0 = t * T
                # gather the tile's T*128 messages, 128 rows per instruction
                msgs = msgp.tile([P, T * f_in], dt.float32)
                for j in range(T):
                    nc.gpsimd.indirect_dma_start(
                        out=msgs[:, j * f_in:(j + 1) * f_in],
                        out_offset=None,
                        in_=table[:, :],
                        in_offset=bass.IndirectOffsetOnAxis(ap=idx_sb[:, c0 + j:c0 + j + 1], axis=0),
                    )
                # selection matrices for all T chunks in one vector op
                S = sp.tile([P, T * P], dt.float32)
                try:
                    nc.vector.tensor_tensor(
                        out=S[:],
                        in0=dr_sb[:, c0:c0 + T, None].to_broadcast([P, T, P]),
                        in1=iota_big[:],
                        op=mybir.AluOpType.is_equal,
                    )
                except Exception:
                    for j in range(T):
                        nc.vector.tensor_tensor(
                            out=S[:, j * P:(j + 1) * P],
                            in0=dr_sb[:, c0 + j:c0 + j + 1].to_broadcast([P, P]),
                            in1=iota_big[:, :P],
                            op=mybir.AluOpType.is_equal,
                        )
                # segment sum: agg[n, f] += S_j^T @ msgs_j
                agg_ps = psA.tile([P, f_in], dt.float32)
                for j in range(T):
                    nc.tensor.matmul(
                        out=agg_ps[:],
                        lhsT=S[:, j * P:(j + 1) * P],
                        rhs=msgs[:, j * f_in:(j + 1) * f_in],
                        start=(j == 0),
                        stop=(j == T - 1),
                    )
                # mean: scale by 1/deg (per-partition scalar), PSUM -> SBUF
                agg_sb = work.tile([P, f_in], dt.float32)
                nc.scalar.mul(agg_sb[:], agg_ps[:], recip[:, t:t + 1])
                # transpose to [f_in, nodes]
                aggT_sb = []
                for si in range(SI):
                    tp = psB.tile([P, P], dt.float32)
                    nc.tensor.transpose(out=tp[:], in_=agg_sb[:, si * P:(si + 1) * P], identity=ident[:])
                    ts = work.tile([P, P], dt.float32)
                    nc.vector.tensor_copy(ts[:], tp[:])
                    aggT_sb.append(ts)
                # dense: zT[fo,n] = sum_si wlT_si^T @ aggT_si + wrT_si^T @ selfT_si
                for so in range(SO):
                    z_ps_full = psC.tile([P, P], dt.float32)
                    z_ps = z_ps_full[:fo_sz, :]
                    nmm = 2 * SI
                    k = 0
                    for si in range(SI):
                        nc.tensor.matmul(
                            out=z_ps[:],
                            lhsT=wl_sb[:, si * f_out + so * fo_sz: si * f_out + so * fo_sz + fo_sz],
                            rhs=aggT_sb[si][:],
                            start=(k == 0), stop=(k == nmm - 1))
                        k += 1
                    for si in range(SI):
                        nc.tensor.matmul(
                            out=z_ps[:],
                            lhsT=wr_sb[:, si * f_out + so * fo_sz: si * f_out + so * fo_sz + fo_sz],
                            rhs=self_sb[si][:, t * P:(t + 1) * P],
                            start=(k == 0), stop=(k == nmm - 1))
                        k += 1
                    o_sb_full = outp.tile([P, P], dt.float32)
                    o_sb = o_sb_full[:fo_sz, :]
                    if relu:
                        nc.scalar.activation(o_sb[:], z_ps[:], mybir.ActivationFunctionType.Relu,
                                             bias=b_sb[:fo_sz, so:so + 1], scale=1.0)
                    else:
                        nc.vector.tensor_add(o_sb[:], z_ps[:], b_sb[:fo_sz, so:so + 1].to_broadcast([fo_sz, P]))
                    nc.sync.dma_start(outT[so * P:so * P + fo_sz, t * P:(t + 1) * P], o_sb[:])
    nc.compile()
    return nc


_PROG_CACHE = {}


def _get_programs(T):
    key = T
    if key not in _PROG_CACHE:
        l1 = _build_layer_program(T, N_NODES, DIM_IN, DIM_H, relu=True)
        l2 = _build_layer_program(T, N_CORES * NPAD_CORE, DIM_H, DIM_OUT, relu=False)
        _PROG_CACHE[key] = (l1, l2)
    return _PROG_CACHE[key]


def _pack_w(w):
    """[f_out, f_in] weight -> [128, SI*f_out] with [p, si*f_out+f] = w[f, si*128+p]."""
    f_out, f_in = w.shape
    si = f_in // P
    return np.ascontiguousarray(np.hstack([w.T[i * P:(i + 1) * P, :] for i in range(si)]), dtype=np.float32)


def _pack_b(b):
    so = (b.shape[0] + P - 1) // P
    out = np.zeros((P, so), np.float32)
    for i in range(so):
        seg = b[i * P:(i + 1) * P]
        out[:seg.shape[0], i] = seg
    return out


def kernel(x, edge_index, W1l, W1r, b1, W2l, W2r, b2):
    global LAST_RESULTS
    LAST_RESULTS = []
    x = np.asarray(x, np.float32)
    src = np.asarray(edge_index[0], np.int64)
    dst = np.asarray(edge_index[1], np.int64)

    deg = np.bincount(dst, minlength=N_NODES)
    tile_of, slot_of, T = _partition_nodes(deg)
    src_cols, dst_cols = _build_edge_layout(src, dst, tile_of, slot_of, T)

    pos_of = tile_of * P + slot_of        # global padded slot (core = tile//49)
    l1, l2 = _get_programs(T)

    trace = bool(int(__import__("os").environ.get("BASS_TRACE", "0") or 0))
    tkw = dict(trace=True, tmpdir=None) if trace else {}

    # per-core metadata
    deg_cols, selfTs = [], []
    for c in range(N_CORES):
        sl = slice(c * TILES_PER_CORE, (c + 1) * TILES_PER_CORE)
        dcol = np.zeros((P, TILES_PER_CORE), np.float32)
        sT = np.zeros((NPAD_CORE, DIM_IN), np.float32)
        tiles = np.arange(*sl.indices(N_TILES)[:2])
        mask = np.isin(tile_of, tiles)
        nodes = np.nonzero(mask)[0]
        local = (tile_of[nodes] - c * TILES_PER_CORE) * P + slot_of[nodes]
        dcol[slot_of[nodes], tile_of[nodes] - c * TILES_PER_CORE] = deg[nodes]
        sT[local] = x[nodes]
        deg_cols.append(dcol)
        selfTs.append(np.ascontiguousarray(sT.T))

    w1l_p, w1r_p, b1_p = _pack_w(np.asarray(W1l)), _pack_w(np.asarray(W1r)), _pack_b(np.asarray(b1))
    w2l_p, w2r_p, b2_p = _pack_w(np.asarray(W2l)), _pack_w(np.asarray(W2r)), _pack_b(np.asarray(b2))

    in_maps = []
    for c in range(N_CORES):
        in_maps.append({
            "table": x,
            "selfT0": selfTs[c],
            "wlT": w1l_p, "wrT": w1r_p, "b_col": b1_p,
            "src_idx": src_cols[c].astype(np.int32),
            "dst_rel": dst_cols[c],
            "deg_col": deg_cols[c],
        })
    r1 = _run_spmd_retry(l1, in_maps, **tkw)
    LAST_RESULTS.append(r1)

    # assemble full h (replicated gather table for layer 2) and per-core selfT
    h_table = np.concatenate([np.ascontiguousarray(r1.results[c]["outT"].T)
                              for c in range(N_CORES)], axis=0)  # [50176, 256]

    src2 = pos_of[src].astype(np.int32)
    src2_cols = []
    for c in range(N_CORES):
        sc = src_cols[c].copy()
        pad = dst_cols[c] == PAD_SLOT
        sc2 = pos_of[sc]
        sc2[pad] = 0
        src2_cols.append(sc2.astype(np.int32))

    in_maps2 = []
    for c in range(N_CORES):
        hT = r1.results[c]["outT"]
        in_maps2.append({
            "table": h_table,
            "selfT0": np.ascontiguousarray(hT[:128]),
            "selfT1": np.ascontiguousarray(hT[128:]),
            "wlT": w2l_p, "wrT": w2r_p, "b_col": b2_p,
            "src_idx": src2_cols[c],
            "dst_rel": dst_cols[c],
            "deg_col": deg_cols[c],
        })
    r2 = _run_spmd_retry(l2, in_maps2, **tkw)
    LAST_RESULTS.append(r2)

    big = np.concatenate([r2.results[c]["outT"] for c in range(N_CORES)], axis=1)  # [64, 50176]
    out = np.ascontiguousarray(big[:, pos_of[np.arange(N_NODES)]].T, dtype=np.float32)
    return out



# revision 7
# speedup vs baseline: 3.2332x; 3.2332x over previous
"""GraphSAGE (2-layer, mean aggregation) on 8 Trainium2 NeuronCores.

Strategy (per spec sharding_hint): destination nodes are sharded across the
8 cores (49 tiles of 128 nodes per core, greedily packed so every tile has
nearly equal incoming-edge counts from each half of the node table). Edge
lists are partitioned by destination tile and padded to uniform per-tile
chunk counts so one SPMD program serves all cores.

Per-edge source rows are fetched with the Pool-engine dma_gather (SWDGE
fast path, ~0.34ns/descriptor) instead of generic indirect DMA
(~9.3ns/descriptor, the previous bottleneck). dma_gather indices are int16,
so the gather table is split at row 32768 into A/B halves and each tile's
edge list is partitioned accordingly; gathers stream over each region in
1024-index instructions (the SWDGE descriptor-carveout limit).

Layer 2 is restructured as transform-then-aggregate: mean aggregation
commutes with the linear maps, so layer 1's program also computes
z = h @ W2l^T and s = h @ W2r^T + b2 (64-dim each) on-device. Layer 2 then
only gathers 64-dim z rows, segment-means them and adds s; h never leaves
the device and layer 2 has no dense matmuls.

The segment sum runs on the PE: a bf16 0/1 selection matrix
S[e, n] = (dst_slot[e] == n) is formed on the vector engine (iota +
is_equal) and S^T @ messages accumulates into fp32 PSUM. Messages and
weights are bf16 (S entries are exact in bf16); all accumulation is fp32.
The host only does integer index preprocessing, sharding/layout (including
the bf16 cast of the replicated x copy), and un-sharding.
"""
import sys
from contextlib import ExitStack

import numpy as np

for _p in ("/opt/trn_rl_repo",):
    if _p not in sys.path:
        sys.path.insert(0, _p)

import concourse.bass as bass
import concourse.tile as tile
from concourse import bacc, mybir, library_config
from concourse.bass_utils import run_bass_kernel_spmd
from concourse.masks import make_identity

try:
    import ml_dtypes
    BF16 = ml_dtypes.bfloat16
except ImportError:  # pragma: no cover
    import jax.numpy as jnp
    BF16 = jnp.bfloat16


def _ensure_axon_hooks():
    """run_bass_kernel_spmd(trace=True) imports antenv.axon_hooks, which this
    image lacks; install a ctypes-backed hook so tracing works (or degrades
    to a no-op instead of an ImportError)."""
    try:
        import antenv.axon_hooks  # noqa: F401
        return
    except ImportError:
        pass
    import contextlib
    import ctypes
    import types

    def _make_hook():
        try:
            lib = ctypes.CDLL("/opt/axon/libaxon_pjrt.so")
        except OSError:
            return None
        if not hasattr(lib, "axon_start_nrt_profile"):
            return None
        lib.axon_start_nrt_profile.argtypes = [ctypes.POINTER(ctypes.c_int64), ctypes.c_size_t]
        lib.axon_start_nrt_profile.restype = ctypes.c_int64
        lib.axon_stop_nrt_profile.argtypes = [ctypes.c_char_p]
        lib.axon_stop_nrt_profile.restype = ctypes.c_int64

        @contextlib.contextmanager
        def _hook(output_dir, device_ids):
            import jax
            jax.devices()
            if device_ids:
                ids = (ctypes.c_int64 * len(device_ids))(*device_ids)
                rc = lib.axon_start_nrt_profile(ids, len(device_ids))
            else:
                rc = lib.axon_start_nrt_profile(None, 0)
            if rc != 0:
                raise RuntimeError(f"axon_start_nrt_profile rc={rc}")
            try:
                yield
            finally:
                lib.axon_stop_nrt_profile(str(output_dir).encode())

        return _hook

    hook = _make_hook()
    mod = types.ModuleType("antenv.axon_hooks")
    mod.get_axon_ntff_profile_hook = lambda: hook
    mod.set_axon_ntff_profile_hook = lambda h: None
    import antenv
    antenv.axon_hooks = mod
    sys.modules["antenv.axon_hooks"] = mod


_ensure_axon_hooks()


def _run_spmd_retry(nc, in_maps, **kw):
    """One retry for transient NRT device errors (axon cores occasionally
    report EXEC_UNIT_UNRECOVERABLE right after a prior faulted run)."""
    import time
    try:
        return run_bass_kernel_spmd(nc, in_maps, core_ids=list(range(N_CORES)), **kw)
    except Exception:
        time.sleep(15)
        return run_bass_kernel_spmd(nc, in_maps, core_ids=list(range(N_CORES)), **kw)


N_NODES = 50000
N_EDGES = 800000
DIM_IN, DIM_H, DIM_OUT = 128, 256, 64
N_CORES = 8
P = 128
TILES_PER_CORE = 49                      # ceil(50000 / 8 / 128)
N_TILES = N_CORES * TILES_PER_CORE       # 392
NPAD_CORE = TILES_PER_CORE * P           # 6272
PAD_SLOT = 200.0                         # dst_rel sentinel: matches no iota lane
SPLIT = 32768                            # int16 idx limit: table A/B boundary
QCH = 8                                  # chunks per dma_gather (1024 idxs)

LAST_RESULTS = []   # test harness reads profiling results from here


def _nchp(T):
    """Padded chunk count for a region with T chunks/tile."""
    n = TILES_PER_CORE * T
    return (n + QCH - 1) // QCH * QCH


def _partition_nodes(degA, degB):
    """Greedily pack nodes into N_TILES bins of <=128 nodes, jointly
    minimizing the max per-bin (sumA, sumB). Returns (tile_of, slot_of,
    TA, TB): uniform A-/B-chunk counts per tile."""
    deg = degA + degB
    order = np.argsort(-deg, kind="stable")
    sumsA = np.zeros(N_TILES, np.float64)
    sumsB = np.zeros(N_TILES, np.float64)
    counts = np.zeros(N_TILES, np.int64)
    tile_of = np.empty(N_NODES, np.int64)
    slot_of = np.empty(N_NODES, np.int64)
    capA = float(degA.sum()) / N_TILES
    capB = float(degB.sum()) / N_TILES
    for node in order:
        dA, dB = degA[node], degB[node]
        score = np.maximum((sumsA + dA) / capA, (sumsB + dB) / capB)
        score[counts >= P] = np.inf
        t = int(np.argmin(score))
        tile_of[node] = t
        slot_of[node] = counts[t]
        counts[t] += 1
        sumsA[t] += dA
        sumsB[t] += dB
    TA = int(np.ceil(sumsA.max() / P))
    TB = int(np.ceil(sumsB.max() / P))
    return tile_of, slot_of, TA, TB


def _wrap_idx(vals):
    """int16 dma_gather idx layout for one core region: flat idx array
    (len multiple of 16) -> [128, len//16], wrapped in 16-partition groups
    and replicated across the 8 groups."""
    n = len(vals)
    cols = vals.reshape(n // 16, 16).T.astype(np.int16)   # [16, n/16]
    return np.ascontiguousarray(np.tile(cols, (8, 1)))    # [128, n/16]


def _build_edge_layout(src, dst, tile_of, slot_of, TA, TB):
    """Per-core edge metadata for the A/B-split gather.

    Chunk-major layout per region R: chunk c of tile t is global chunk
    t*TR + c; lane p of that chunk is edge c*128+p of the tile's padded
    R-edge list. Regions are padded to a QCH-multiple of chunks.
    Returns per-core lists: idxA int16 [128, NCHP_A*8], idxB, dstA bf16
    [128, NCHP_A] (dst slot per edge lane, PAD_SLOT on padding), dstB.
    """
    etile = tile_of[dst]
    isA = src < SPLIT
    nA, nB = TA * P, TB * P
    idxA_pad = np.zeros((N_TILES, nA), np.int64)
    idxB_pad = np.zeros((N_TILES, nB), np.int64)
    dstA_pad = np.full((N_TILES, nA), PAD_SLOT, np.float32)
    dstB_pad = np.full((N_TILES, nB), PAD_SLOT, np.float32)
    for idx_pad, dst_pad, mask, base in (
            (idxA_pad, dstA_pad, isA, 0), (idxB_pad, dstB_pad, ~isA, SPLIT)):
        s, d, et = src[mask], dst[mask], etile[mask]
        order = np.argsort(et, kind="stable")
        s, d, et = s[order], d[order], et[order]
        counts = np.bincount(et, minlength=N_TILES)
        starts = np.concatenate([[0], np.cumsum(counts)[:-1]])
        rank = np.arange(len(s)) - np.repeat(starts, counts)
        idx_pad[et, rank] = s - base
        dst_pad[et, rank] = slot_of[d]
    NCHP_A, NCHP_B = _nchp(TA), _nchp(TB)
    idxA, idxB, dstA, dstB = [], [], [], []
    for c in range(N_CORES):
        sl = slice(c * TILES_PER_CORE, (c + 1) * TILES_PER_CORE)
        for idx_pad, dst_pad, T, NCHP, idx_l, dst_l in (
                (idxA_pad, dstA_pad, TA, NCHP_A, idxA, dstA),
                (idxB_pad, dstB_pad, TB, NCHP_B, idxB, dstB)):
            flat_idx = np.zeros(NCHP * P, np.int64)
            flat_idx[:TILES_PER_CORE * T * P] = idx_pad[sl].reshape(-1)
            idx_l.append(_wrap_idx(flat_idx))
            dcols = np.full((P, NCHP), PAD_SLOT, np.float32)
            dcols[:, :TILES_PER_CORE * T] = (
                dst_pad[sl].reshape(TILES_PER_CORE * T, P).T)
            dst_l.append(np.ascontiguousarray(dcols.astype(BF16)))
    return idxA, idxB, dstA, dstB


def _common_inputs(nc, TA, TB):
    dt = mybir.dt
    NCHP_A, NCHP_B = _nchp(TA), _nchp(TB)
    t = {}
    t["idxA"] = nc.dram_tensor("idxA", [P, NCHP_A * 8], dt.int16,
                               kind="ExternalInput").ap()
    t["idxB"] = nc.dram_tensor("idxB", [P, NCHP_B * 8], dt.int16,
                               kind="ExternalInput").ap()
    t["dstA"] = nc.dram_tensor("dstA", [P, NCHP_A], dt.bfloat16,
                               kind="ExternalInput").ap()
    t["dstB"] = nc.dram_tensor("dstB", [P, NCHP_B], dt.bfloat16,
                               kind="ExternalInput").ap()
    t["deg_col"] = nc.dram_tensor("deg_col", [P, TILES_PER_CORE], dt.float32,
                                  kind="ExternalInput").ap()
    return t


def _load_common(nc, tc, const, t, TA, TB):
    """Load shared SBUF constants; returns dict of SBUF tiles."""
    dt = mybir.dt
    NCHP_A, NCHP_B = _nchp(TA), _nchp(TB)
    s = {}
    s["idxA"] = const.tile([P, NCHP_A * 8], dt.int16, name="idxA_sb")
    nc.sync.dma_start(s["idxA"][:], t["idxA"][:, :])
    s["idxB"] = const.tile([P, NCHP_B * 8], dt.int16, name="idxB_sb")
    nc.sync.dma_start(s["idxB"][:], t["idxB"][:, :])
    s["dstA"] = const.tile([P, NCHP_A], dt.bfloat16, name="dstA_sb")
    nc.sync.dma_start(s["dstA"][:], t["dstA"][:, :])
    s["dstB"] = const.tile([P, NCHP_B], dt.bfloat16, name="dstB_sb")
    nc.sync.dma_start(s["dstB"][:], t["dstB"][:, :])
    deg_sb = const.tile([P, TILES_PER_CORE], dt.float32)
    nc.sync.dma_start(deg_sb[:], t["deg_col"][:, :])

    iota_f = const.tile([P, P], dt.float32)
    nc.gpsimd.iota(iota_f[:], pattern=[[1, P]], base=0, channel_multiplier=0,
                   allow_small_or_imprecise_dtypes=True)
    iota_sm = const.tile([P, P], dt.bfloat16)
    nc.vector.tensor_copy(iota_sm[:], iota_f[:])
    iota_big = const.tile([P, QCH * P], dt.bfloat16)
    for _j in range(QCH):
        nc.vector.tensor_copy(iota_big[:, _j * P:(_j + 1) * P], iota_sm[:])
    s["iota"] = iota_big

    recip = const.tile([P, TILES_PER_CORE], dt.float32)
    nc.vector.tensor_scalar_max(recip[:], deg_sb[:], 1.0)
    nc.vector.reciprocal(recip[:], recip[:])
    s["recip"] = recip
    return s


def _make_gather_streams(nc, sb, table_f, msg_pools, s_pools, TA, TB, fdim, dtyp):
    """Returns an `ensure(region, chunk)` closure that lazily issues the
    1024-idx dma_gather + S-build for the gather group covering `chunk`,
    plus accessors msgs(region, chunk) -> (tile, slot) and S blocks."""
    dt = mybir.dt
    state = {"A": {"next": 0, "msgs": {}, "S": {}},
             "B": {"next": 0, "msgs": {}, "S": {}}}
    cfg = {"A": (sb["idxA"], sb["dstA"], table_f["A"], TA),
           "B": (sb["idxB"], sb["dstB"], table_f["B"], TB)}

    def issue(region, g):
        idx_sb, dst_sb, tbl, _T = cfg[region]
        st = state[region]
        msgs = msg_pools[region].tile([P, QCH, fdim], dtyp,
                                      name=f"msgs{region}")
        nc.gpsimd.dma_gather(
            msgs[:], tbl,
            idx_sb[:, g * QCH * 8:(g + 1) * QCH * 8],
            QCH * P, QCH * P, fdim)
        S = s_pools[region].tile([P, QCH * P], dt.bfloat16,
                                 name=f"S{region}")
        try:
            nc.vector.tensor_tensor(
                out=S[:],
                in0=dst_sb[:, g * QCH:(g + 1) * QCH, None].to_broadcast(
                    [P, QCH, P]),
                in1=sb["iota"][:],
                op=mybir.AluOpType.is_equal)
        except Exception:
            for j in range(QCH):
                nc.vector.tensor_tensor(
                    out=S[:, j * P:(j + 1) * P],
                    in0=dst_sb[:, g * QCH + j:g * QCH + j + 1].to_broadcast([P, P]),
                    in1=sb["iota"][:, :P],
                    op=mybir.AluOpType.is_equal)
        st["msgs"][g] = msgs
        st["S"][g] = S

    def ensure(region, chunk):
        g = chunk // QCH
        st = state[region]
        while st["next"] <= g:
            issue(region, st["next"])
            st["next"] += 1
        return st["msgs"][g], st["S"][g], chunk % QCH

    return ensure


def _build_l1_program(TA, TB):
    """Layer 1 + fused layer-2 pre-transforms as one SPMD bass program."""
    dt = mybir.dt
    nc = bacc.Bacc("TRN2", target_bir_lowering=False, debug=False,
                   enable_asserts=False, num_devices=N_CORES)
    table = nc.dram_tensor("table", [N_NODES, DIM_IN], dt.bfloat16,
                           kind="ExternalInput").ap()
    selfT = nc.dram_tensor("selfT", [P, NPAD_CORE], dt.bfloat16,
                           kind="ExternalInput").ap()
    com = _common_inputs(nc, TA, TB)
    w1l = nc.dram_tensor("w1l", [P, DIM_H], dt.bfloat16, kind="ExternalInput").ap()
    w1r = nc.dram_tensor("w1r", [P, DIM_H], dt.bfloat16, kind="ExternalInput").ap()
    b1 = nc.dram_tensor("b1", [P, 2], dt.float32, kind="ExternalInput").ap()
    w2l = nc.dram_tensor("w2l", [P, 2 * DIM_OUT], dt.bfloat16, kind="ExternalInput").ap()
    w2r = nc.dram_tensor("w2r", [P, 2 * DIM_OUT], dt.bfloat16, kind="ExternalInput").ap()
    b2 = nc.dram_tensor("b2", [1, DIM_OUT], dt.bfloat16, kind="ExternalInput").ap()
    z_out = nc.dram_tensor("z_out", [NPAD_CORE, DIM_OUT], dt.float32,
                           kind="ExternalOutput").ap()
    s_out = nc.dram_tensor("s_out", [NPAD_CORE, DIM_OUT], dt.float32,
                           kind="ExternalOutput").ap()

    with tile.TileContext(nc) as tc:
        with ExitStack() as ctx:
            const = ctx.enter_context(tc.tile_pool(name="const", bufs=1))
            msgpA = ctx.enter_context(tc.tile_pool(name="msgpA", bufs=4))
            msgpB = ctx.enter_context(tc.tile_pool(name="msgpB", bufs=4))
            spA = ctx.enter_context(tc.tile_pool(name="spA", bufs=4))
            spB = ctx.enter_context(tc.tile_pool(name="spB", bufs=4))
            work = ctx.enter_context(tc.tile_pool(name="work", bufs=3))
            outp = ctx.enter_context(tc.tile_pool(name="outp", bufs=4))
            psA = ctx.enter_context(tc.tile_pool(name="psA", bufs=2, space="PSUM"))
            psT = ctx.enter_context(tc.tile_pool(name="psT", bufs=1, space="PSUM"))
            psH = ctx.enter_context(tc.tile_pool(name="psH", bufs=2, space="PSUM"))
            psZ = ctx.enter_context(tc.tile_pool(name="psZ", bufs=2, space="PSUM"))

            sb = _load_common(nc, tc, const, com, TA, TB)
            w1l_sb = const.tile([P, DIM_H], dt.bfloat16)
            nc.sync.dma_start(w1l_sb[:], w1l[:, :])
            w1r_sb = const.tile([P, DIM_H], dt.bfloat16)
            nc.sync.dma_start(w1r_sb[:], w1r[:, :])
            b1_sb = const.tile([P, 2], dt.float32)
            nc.sync.dma_start(b1_sb[:], b1[:, :])
            w2l_sb = const.tile([P, 2 * DIM_OUT], dt.bfloat16)
            nc.sync.dma_start(w2l_sb[:], w2l[:, :])
            w2r_sb = const.tile([P, 2 * DIM_OUT], dt.bfloat16)
            nc.sync.dma_start(w2r_sb[:], w2r[:, :])
            b2_sb = const.tile([1, DIM_OUT], dt.bfloat16)
            nc.sync.dma_start(b2_sb[:], b2[:, :])
            self_sb = const.tile([P, NPAD_CORE], dt.bfloat16)
            nc.sync.dma_start(self_sb[:], selfT[:, :])

            ident = const.tile([P, P], dt.bfloat16)
            make_identity(nc, ident[:])
            ones1 = const.tile([1, P], dt.bfloat16)
            nc.gpsimd.memset(ones1[:], 1.0)

            ensure = _make_gather_streams(
                nc, sb, {"A": table[:SPLIT, :], "B": table[SPLIT:, :]},
                {"A": msgpA, "B": msgpB}, {"A": spA, "B": spB},
                TA, TB, DIM_IN, dt.bfloat16)

            for t in range(TILES_PER_CORE):
                # segment sum: agg[n, f] += S_j^T @ msgs_j over A then B chunks
                agg_ps = psA.tile([P, DIM_IN], dt.float32)
                nmm = TA + TB
                k = 0
                for region, T in (("A", TA), ("B", TB)):
                    for j in range(T):
                        msgs, S, slot = ensure(region, t * T + j)
                        nc.tensor.matmul(
                            out=agg_ps[:],
                            lhsT=S[:, slot * P:(slot + 1) * P],
                            rhs=msgs[:, slot, :],
                            start=(k == 0), stop=(k == nmm - 1))
                        k += 1
                # mean (per-partition scalar 1/deg), PSUM -> SBUF bf16
                agg_sb = work.tile([P, DIM_IN], dt.bfloat16)
                nc.scalar.mul(agg_sb[:], agg_ps[:], sb["recip"][:, t:t + 1])
                # transpose to [f_in, nodes]
                tp = psT.tile([P, P], dt.bfloat16)
                nc.tensor.transpose(out=tp[:], in_=agg_sb[:], identity=ident[:])
                aggT_sb = work.tile([P, P], dt.bfloat16)
                nc.vector.tensor_copy(aggT_sb[:], tp[:])
                # dense: hT[fo_block, n] = w1l_so^T @ aggT + w1r_so^T @ xT
                hT_sb = work.tile([P, 2, P], dt.bfloat16)
                for so in range(2):
                    h_ps = psH.tile([P, P], dt.float32)
                    nc.tensor.matmul(
                        out=h_ps[:], lhsT=w1l_sb[:, so * P:(so + 1) * P],
                        rhs=aggT_sb[:], start=True, stop=False)
                    nc.tensor.matmul(
                        out=h_ps[:], lhsT=w1r_sb[:, so * P:(so + 1) * P],
                        rhs=self_sb[:, t * P:(t + 1) * P],
                        start=False, stop=True)
                    nc.scalar.activation(
                        hT_sb[:, so, :], h_ps[:],
                        mybir.ActivationFunctionType.Relu,
                        bias=b1_sb[:, so:so + 1], scale=1.0)
                # layer-2 pre-transforms, node-major: z = h @ W2l^T,
                # s = h @ W2r^T + b2  (bias via rank-1 ones x b2)
                z_ps = psZ.tile([P, DIM_OUT], dt.float32, name="zs_ps")
                for si in range(2):
                    nc.tensor.matmul(
                        out=z_ps[:], lhsT=hT_sb[:, si, :],
                        rhs=w2l_sb[:, si * DIM_OUT:(si + 1) * DIM_OUT],
                        start=(si == 0), stop=(si == 1))
                z_sb = outp.tile([P, DIM_OUT], dt.float32)
                nc.scalar.mul(z_sb[:], z_ps[:], 1.0)
                nc.sync.dma_start(z_out[t * P:(t + 1) * P, :], z_sb[:])
                s_ps = psZ.tile([P, DIM_OUT], dt.float32, name="zs_ps")
                for si in range(2):
                    nc.tensor.matmul(
                        out=s_ps[:], lhsT=hT_sb[:, si, :],
                        rhs=w2r_sb[:, si * DIM_OUT:(si + 1) * DIM_OUT],
                        start=(si == 0), stop=False)
                nc.tensor.matmul(
                    out=s_ps[:], lhsT=ones1[:], rhs=b2_sb[:],
                    start=False, stop=True)
                s_sb = outp.tile([P, DIM_OUT], dt.float32)
                nc.scalar.mul(s_sb[:], s_ps[:], 1.0)
                nc.sync.dma_start(s_out[t * P:(t + 1) * P, :], s_sb[:])
    nc.compile()
    return nc


def _build_l2_program(TA, TB):
    """Layer 2: gather z rows, segment-mean, add s."""
    dt = mybir.dt
    nc = bacc.Bacc("TRN2", target_bir_lowering=False, debug=False,
                   enable_asserts=False, num_devices=N_CORES)
    ztab = nc.dram_tensor("ztab", [N_NODES, DIM_OUT], dt.float32,
                          kind="ExternalInput").ap()
    s_in = nc.dram_tensor("s_in", [NPAD_CORE, DIM_OUT], dt.float32,
                          kind="ExternalInput").ap()
    com = _common_inputs(nc, TA, TB)
    out = nc.dram_tensor("out", [NPAD_CORE, DIM_OUT], dt.float32,
                         kind="ExternalOutput").ap()

    with tile.TileContext(nc) as tc:
        with ExitStack() as ctx:
            const = ctx.enter_context(tc.tile_pool(name="const", bufs=1))
            msgpA = ctx.enter_context(tc.tile_pool(name="msgpA", bufs=4))
            msgpB = ctx.enter_context(tc.tile_pool(name="msgpB", bufs=4))
            mbfA = ctx.enter_context(tc.tile_pool(name="mbfA", bufs=4))
            mbfB = ctx.enter_context(tc.tile_pool(name="mbfB", bufs=4))
            spA = ctx.enter_context(tc.tile_pool(name="spA", bufs=4))
            spB = ctx.enter_context(tc.tile_pool(name="spB", bufs=4))
            sload = ctx.enter_context(tc.tile_pool(name="sload", bufs=3))
            outp = ctx.enter_context(tc.tile_pool(name="outp", bufs=4))
            psA = ctx.enter_context(tc.tile_pool(name="psA", bufs=2, space="PSUM"))

            sb = _load_common(nc, tc, const, com, TA, TB)

            # gather fp32 z rows, then cast each group to bf16 on the scalar
            # engine; the bf16 copies are what the matmuls consume.
            raw_ensure = _make_gather_streams(
                nc, sb, {"A": ztab[:SPLIT, :], "B": ztab[SPLIT:, :]},
                {"A": msgpA, "B": msgpB}, {"A": spA, "B": spB},
                TA, TB, DIM_OUT, dt.float32)
            bf_pools = {"A": mbfA, "B": mbfB}
            bf_cache = {"A": {}, "B": {}}

            def ensure_bf(region, chunk):
                msgs, S, slot = raw_ensure(region, chunk)
                g = chunk // QCH
                if g not in bf_cache[region]:
                    mbf = bf_pools[region].tile([P, QCH, DIM_OUT], dt.bfloat16,
                                                name=f"mbf{region}")
                    nc.scalar.mul(mbf[:], msgs[:], 1.0)
                    bf_cache[region][g] = mbf
                return bf_cache[region][g], S, slot

            for t in range(TILES_PER_CORE):
                agg_ps = psA.tile([P, DIM_OUT], dt.float32)
                nmm = TA + TB
                k = 0
                for region, T in (("A", TA), ("B", TB)):
                    for j in range(T):
                        mbf, S, slot = ensure_bf(region, t * T + j)
                        nc.tensor.matmul(
                            out=agg_ps[:],
                            lhsT=S[:, slot * P:(slot + 1) * P],
                            rhs=mbf[:, slot, :],
                            start=(k == 0), stop=(k == nmm - 1))
                        k += 1
                s_tile = sload.tile([P, DIM_OUT], dt.float32)
                nc.sync.dma_start(s_tile[:], s_in[t * P:(t + 1) * P, :])
                agg_sb = outp.tile([P, DIM_OUT], dt.float32)
                nc.scalar.mul(agg_sb[:], agg_ps[:], sb["recip"][:, t:t + 1])
                o_sb = outp.tile([P, DIM_OUT], dt.float32)
                nc.vector.tensor_add(o_sb[:], agg_sb[:], s_tile[:])
                nc.sync.dma_start(out[t * P:(t + 1) * P, :], o_sb[:])
    nc.compile()
    return nc


_PROG_CACHE = {}


def _get_programs(TA, TB):
    key = (TA, TB)
    if key not in _PROG_CACHE:
        _PROG_CACHE[key] = (_build_l1_program(TA, TB), _build_l2_program(TA, TB))
    return _PROG_CACHE[key]


def kernel(x, edge_index, W1l, W1r, b1, W2l, W2r, b2):
    global LAST_RESULTS
    LAST_RESULTS = []
    x = np.asarray(x, np.float32)
    src = np.asarray(edge_index[0], np.int64)
    dst = np.asarray(edge_index[1], np.int64)

    deg = np.bincount(dst, minlength=N_NODES)
    degA = np.bincount(dst[src < SPLIT], minlength=N_NODES)
    degB = deg - degA
    tile_of, slot_of, TA, TB = _partition_nodes(degA, degB)
    idxA, idxB, dstA, dstB = _build_edge_layout(src, dst, tile_of, slot_of, TA, TB)

    l1, l2 = _get_programs(TA, TB)

    trace = bool(int(__import__("os").environ.get("BASS_TRACE", "0") or 0))
    tkw = dict(trace=True, tmpdir=None) if trace else {}

    x_bf = x.astype(BF16)
    deg_cols, selfTs, node_lists, local_lists = [], [], [], []
    for c in range(N_CORES):
        tiles = np.arange(c * TILES_PER_CORE, (c + 1) * TILES_PER_CORE)
        mask = np.isin(tile_of, tiles)
        nodes = np.nonzero(mask)[0]
        local_tile = tile_of[nodes] - c * TILES_PER_CORE
        local = local_tile * P + slot_of[nodes]
        dcol = np.zeros((P, TILES_PER_CORE), np.float32)
        dcol[slot_of[nodes], local_tile] = deg[nodes]
        sT = np.zeros((NPAD_CORE, DIM_IN), BF16)
        sT[local] = x_bf[nodes]
        deg_cols.append(dcol)
        selfTs.append(np.ascontiguousarray(sT.T))
        node_lists.append(nodes)
        local_lists.append(local)

    w1l_p = np.ascontiguousarray(np.asarray(W1l, np.float32).T).astype(BF16)
    w1r_p = np.ascontiguousarray(np.asarray(W1r, np.float32).T).astype(BF16)
    b1_np = np.asarray(b1, np.float32)
    b1_p = np.stack([b1_np[:P], b1_np[P:]], axis=1).astype(np.float32)
    # w2 packed [p, si*64+f] = W2[f, si*128+p]
    w2l_np = np.asarray(W2l, np.float32)     # [64, 256]
    w2r_np = np.asarray(W2r, np.float32)
    w2l_p = np.ascontiguousarray(np.hstack([w2l_np.T[:P, :], w2l_np.T[P:, :]])).astype(BF16)
    w2r_p = np.ascontiguousarray(np.hstack([w2r_np.T[:P, :], w2r_np.T[P:, :]])).astype(BF16)
    b2_p = np.asarray(b2, np.float32)[None, :].astype(BF16)

    in_maps = []
    for c in range(N_CORES):
        in_maps.append({
            "table": x_bf,
            "selfT": selfTs[c],
            "idxA": idxA[c], "idxB": idxB[c],
            "dstA": dstA[c], "dstB": dstB[c],
            "deg_col": deg_cols[c],
            "w1l": w1l_p, "w1r": w1r_p, "b1": b1_p,
            "w2l": w2l_p, "w2r": w2r_p, "b2": b2_p,
        })
    r1 = _run_spmd_retry(l1, in_maps, **tkw)
    LAST_RESULTS.append(r1)

    # assemble the replicated z table in ORIGINAL node-id order (so layer 2
    # reuses the same idx arrays); s stays per-core
    z_full = np.zeros((N_NODES, DIM_OUT), np.float32)
    for c in range(N_CORES):
        z_full[node_lists[c]] = r1.results[c]["z_out"][local_lists[c]]

    in_maps2 = []
    for c in range(N_CORES):
        in_maps2.append({
            "ztab": z_full,
            "s_in": r1.results[c]["s_out"],
            "idxA": idxA[c], "idxB": idxB[c],
            "dstA": dstA[c], "dstB": dstB[c],
            "deg_col": deg_cols[c],
        })
    r2 = _run_spmd_retry(l2, in_maps2, **tkw)
    LAST_RESULTS.append(r2)

    out = np.zeros((N_NODES, DIM_OUT), np.float32)
    for c in range(N_CORES):
        out[node_lists[c]] = r2.results[c]["out"][local_lists[c]]
    return out
